# revision 6
# baseline (speedup 1.0000x reference)
"""DIEN model Trainium2 kernel (8-core SPMD, batch-sharded) — v2.

Model (per reference): B=2048, S=200, D=H=ATT=64.
  1. Interest-extraction GRU over time.
  2. Concat-MLP attention + masked softmax over time.
  3. Attentional GRU (AGRU) scan -> final hidden (B, H).

v2 changes over the v1 baseline:
  * bf16 matmul inputs everywhere (weights, x, h, t); fp32 PSUM accum.
    Validated end-to-end in numpy: max rel err 6.5e-3 vs 2e-2 budget.
  * float32r (bit-identical to fp32) for transposes and attention-weight
    tensors: 1.5 cyc/row transposes instead of 2.
  * Batch globally sorted by length (descending), round-robin sharded
    across cores so all cores see the same length profile.  Per-step
    active width c_s = #cols with len > s (rounded up to 32): all scan
    ops are sliced to [*, 0:c_s].  Frozen columns are never read in a
    way that matters (attention scores masked, AGRU a == 0).
  * GRU combine restructured as h' = p + w with q = zbar*h, p = h - q
    computed off the critical path right after sigmoid, and only
    w = zbar*n, h' = p + w after tanh (2 on-path DVE ops, was 3).
  * AGRU: z/r gates computed in separate 64-partition PSUM tiles so all
    elementwise ops are partition-aligned (no Pool copies); attention
    weight broadcast via PE ones-matmul instead of gpsimd.
"""

import os
import numpy as np
import ml_dtypes

B, S, D, H, ATT = 2048, 200, 64, 64, 64
NCORES = 8
BS = B // NCORES          # 256 batch rows per core
BT = BS // 128            # 2 batch tiles of 128

_CACHE = {}

bfloat16 = ml_dtypes.bfloat16


def _build_program(npos, widths):
    import concourse.bass as bass
    import concourse.mybir as mybir
    from concourse import bacc
    from concourse.tile import TileContext

    fp32 = mybir.dt.float32
    f32r = mybir.dt.float32r
    bf16 = mybir.dt.bfloat16
    AF = mybir.ActivationFunctionType
    OP = mybir.AluOpType
    AX = mybir.AxisListType

    nc = bacc.Bacc(None, target_bir_lowering=False)

    # ---------------- DRAM I/O ----------------
    beh = nc.dram_tensor("behavior", [BS, S, D], f32r, kind="ExternalInput")
    tgt = nc.dram_tensor("target", [BS, D], f32r, kind="ExternalInput")
    lens = nc.dram_tensor("lengths_f", [BS, 1], fp32, kind="ExternalInput")
    wihT = nc.dram_tensor("wihT", [128, 3 * H], bf16, kind="ExternalInput")   # dup row halves
    whhT = nc.dram_tensor("whhT", [128, 3 * H], bf16, kind="ExternalInput")   # dup row halves
    a1sT = nc.dram_tensor("a1sT", [H + D, ATT], bf16, kind="ExternalInput")
    w4iT = nc.dram_tensor("w4iT", [H, 3 * H], bf16, kind="ExternalInput")     # [z|r|n] input parts
    w4hT = nc.dram_tensor("w4hT", [H, 3 * H], bf16, kind="ExternalInput")     # [z|r|n] hidden parts
    identr = nc.dram_tensor("identr", [128, 128], f32r, kind="ExternalInput")
    identb = nc.dram_tensor("identb", [128, 128], bf16, kind="ExternalInput")
    iota_r = nc.dram_tensor("iota_r", [1, S], fp32, kind="ExternalInput")
    ones64 = nc.dram_tensor("ones64", [1, H], f32r, kind="ExternalInput")
    svec_d = nc.dram_tensor("svec", [128, 1], fp32, kind="ExternalInput")     # +1 x64, -1 x64
    bias2 = nc.dram_tensor("bias2", [128, 1], fp32, kind="ExternalInput")     # (bih+bhh)[r], -(..)[z]
    biasn = nc.dram_tensor("biasn", [128, 2], fp32, kind="ExternalInput")     # [0:64,0]=bih_n ; [64:128,1]=bhh_n
    bias4 = nc.dram_tensor("bias4", [H, 3], fp32, kind="ExternalInput")       # cols: bz | br | bn

    hout = nc.dram_tensor("h_out", [H, BS], bf16, kind="ExternalOutput")

    # DRAM scratch
    outs_d = nc.dram_tensor("outs_d", [S, H, BS], bf16)
    att_d = nc.dram_tensor("att_d", [S, BS], fp32)

    def nbt(c):  # active 128-col tiles
        return (c + 127) // 128

    with TileContext(nc) as tc:
        with (
            tc.tile_pool(name="const", bufs=1) as cpool,
            tc.tile_pool(name="stage", bufs=6) as stage,
            tc.tile_pool(name="xt", bufs=6) as xtp,
            tc.tile_pool(name="hip", bufs=3) as hip,
            tc.tile_pool(name="ew", bufs=4) as ew,
            tc.tile_pool(name="relu", bufs=2) as relup,
            tc.tile_pool(name="psA", bufs=1, space="PSUM") as psA,   # prz/pn/phn0/phn1
            tc.tile_pool(name="ps3", bufs=1, space="PSUM") as ps3,   # p3 attention
        ):
            # ---------------- constants into SBUF ----------------
            def cload(name, dram, shape, dt_):
                t = cpool.tile(shape, dt_, tag=name)
                nc.sync.dma_start(t[:], dram[:])
                return t

            wih_s = cload("wih", wihT, [128, 3 * H], bf16)
            whh_s = cload("whh", whhT, [128, 3 * H], bf16)
            a1_s = cload("a1", a1sT, [H + D, ATT], bf16)
            w4i_s = cload("w4i", w4iT, [H, 3 * H], bf16)
            w4h_s = cload("w4h", w4hT, [H, 3 * H], bf16)
            idr_s = cload("idr", identr, [128, 128], f32r)
            idb_s = cload("idb", identb, [128, 128], bf16)
            iota_s = cload("iota", iota_r, [1, S], fp32)
            ones_s = cload("ones", ones64, [1, H], f32r)
            svec_s = cload("svec", svec_d, [128, 1], fp32)
            bias2_s = cload("bias2", bias2, [128, 1], fp32)
            biasn_s = cload("biasn", biasn, [128, 2], fp32)
            bias4_s = cload("bias4", bias4, [H, 3], fp32)
            lens_s = cpool.tile([128, BT], fp32, tag="lens")
            for bt in range(BT):
                nc.sync.dma_start(lens_s[:, bt : bt + 1], lens[bt * 128 : (bt + 1) * 128, :])

            # h ping-pong buffers [128, BS] bf16: rows 0:64 = h, rows 64:128 = target^T
            hbuf = [cpool.tile([128, BS], bf16, tag=f"hbuf{i}", name=f"hbuf{i}") for i in range(2)]

            for bt in range(BT):
                tg_st = stage.tile([128, D], f32r, tag="tgst")
                nc.sync.dma_start(tg_st[:], tgt[bt * 128 : (bt + 1) * 128, :])
                pt = psA.tile([128, BS], fp32, tag="phn0", name="pt")
                nc.tensor.transpose(pt[0:D, 0:128].bitcast(f32r), tg_st[:], idr_s[:])
                tg_ev = stage.tile([D, 128], bf16, tag="tgev")
                nc.scalar.copy(tg_ev[:], pt[0:D, 0:128])
                for i in range(2):
                    nc.gpsimd.tensor_copy(
                        out=hbuf[i][64:128, bt * 128 : (bt + 1) * 128], in_=tg_ev[:]
                    )
            nc.vector.memset(hbuf[0][0:64, :], 0.0)

            scores = [cpool.tile([128, S], fp32, tag=f"sc{bt}", name=f"sc{bt}") for bt in range(BT)]
            for bt in range(BT):
                nc.vector.memset(scores[bt][:], 0.0)

            # =========== PHASE 2: GRU scan (+ fused attention MLP) ===========
            xt = None
            p3 = None
            n1 = 0
            for s in range(S):
                c = widths[s]
                hp = hbuf[s % 2]          # h_{s-1} in rows 0:64
                hn_buf = hbuf[(s + 1) % 2]

                # ---- x^T for this step (2 steps per PE transpose) ----
                if s % 2 == 0:
                    xt = xtp.tile([128, BS], bf16, tag="xt")
                    pxt = psA.tile([128, BS], fp32, tag="phn1", name="pxt")
                    for bt in range(nbt(c)):
                        bst = stage.tile([128, 128], f32r, tag="bst")
                        nc.sync.dma_start(
                            bst[:],
                            beh[bt * 128 : (bt + 1) * 128, s : s + 2, :].rearrange(
                                "b s d -> b (s d)"
                            ),
                        )
                        nc.tensor.transpose(
                            pxt[:, bt * 128 : (bt + 1) * 128].bitcast(f32r), bst[:], idr_s[:])
                        nc.scalar.copy(xt[:, bt * 128 : (bt + 1) * 128],
                                       pxt[:, bt * 128 : (bt + 1) * 128])
                half = s % 2
                x_s = xt[half * 64 : half * 64 + 64, 0:c]
                tp_x = (half * 64, 0)
                wih_rows = wih_s[half * 64 : half * 64 + 64, :]

                # ---- gate pre-activations, full active width ----
                p_rz = psA.tile([128, BS], fp32, tag="prz", name="prz")
                nc.tensor.matmul(
                    p_rz[:, 0:c], wih_rows[:, 0:128], x_s,
                    start=True, stop=False, tile_position=tp_x,
                )
                nc.tensor.matmul(
                    p_rz[:, 0:c], whh_s[0:64, 0:128], hp[0:64, 0:c],
                    start=False, stop=True, tile_position=(0, 0),
                )
                p_hn = psA.tile([128, BS], fp32, tag="phn0", name="phn")
                nc.tensor.matmul(
                    p_hn[64:128, 0:c], whh_s[0:64, 128:192], hp[0:64, 0:c],
                    start=True, stop=True, tile_position=(0, 64),
                )
                p_n = psA.tile([H, BS], fp32, tag="pn", name="pn")
                nc.tensor.matmul(
                    p_n[:, 0:c], wih_rows[:, 128:192], x_s,
                    start=True, stop=False, tile_position=tp_x,
                )

                # sigma: rows 0:64 = 1-z, rows 64:128 = r
                rz = ew.tile([128, BS], fp32, tag="rz", name="rz")
                nc.scalar.activation(rz[:, 0:c], p_rz[:, 0:c], AF.Sigmoid,
                                     bias=bias2_s[:], scale=svec_s[:])

                # off-path: q = zbar*h, p = h - q
                q_t = ew.tile([H, BS], fp32, tag="qt", name="qt")
                nc.vector.tensor_tensor(q_t[:, 0:c], rz[0:64, 0:c], hp[0:64, 0:c], OP.mult)
                pp_t = ew.tile([H, BS], fp32, tag="ppt", name="ppt")
                nc.vector.tensor_tensor(pp_t[:, 0:c], hp[0:64, 0:c], q_t[:, 0:c], OP.subtract)

                # n-gate: t = (hn + bhh_n) * r on rows 64:128, accum into p_n via PE
                t_t = ew.tile([128, BS], bf16, tag="tt", name="tt")
                nc.vector.scalar_tensor_tensor(
                    t_t[64:128, 0:c], p_hn[64:128, 0:c], biasn_s[64:128, 1:2], rz[64:128, 0:c],
                    op0=OP.add, op1=OP.mult,
                )
                nc.tensor.matmul(
                    p_n[:, 0:c], idb_s[64:128, 64:128], t_t[64:128, 0:c],
                    start=False, stop=True, tile_position=(64, 0),
                )
                n_t = ew.tile([H, BS], fp32, tag="nt", name="nt")
                nc.scalar.activation(n_t[:, 0:c], p_n[:, 0:c], AF.Tanh, bias=biasn_s[0:64, 0:1])

                # on-path combine: w = zbar*n ; h' = p + w  (bf16 out)
                w_t = ew.tile([H, BS], fp32, tag="wt", name="wt")
                nc.vector.tensor_tensor(w_t[:, 0:c], rz[0:64, 0:c], n_t[:, 0:c], OP.mult)
                nc.vector.tensor_tensor(hn_buf[0:64, 0:c], pp_t[:, 0:c], w_t[:, 0:c], OP.add)

                nc.sync.dma_start(outs_d[s, :, 0:c], hn_buf[0:64, 0:c])

                # ---- fused attention MLP (batch-major, per 128-col tile) ----
                slot = s % 8
                if slot == 0:
                    base_bt = nbt(c)
                    p3 = [ps3.tile([128, 512], fp32, tag=f"p3_{bt}", name=f"p3_{bt}")
                          for bt in range(base_bt)]
                    n1 = 0
                for bt in range(min(len(p3), nbt(c))):
                    nc.tensor.matmul(
                        p3[bt][:, slot * 64 : slot * 64 + 64],
                        hn_buf[:, bt * 128 : (bt + 1) * 128], a1_s[:],
                        start=True, stop=True, tile_position=(0, 0),
                    )
                if nbt(c) > 1:
                    n1 = slot + 1
                if slot == 7 or s == S - 1:
                    ns = slot + 1
                    base = s - slot
                    for bt in range(len(p3)):
                        nsb = ns if bt == 0 else n1
                        if nsb == 0:
                            continue
                        rb = relup.tile([128, 512], fp32, tag=f"rb{bt}")
                        nc.scalar.activation(rb[:, 0 : nsb * 64], p3[bt][:, 0 : nsb * 64], AF.Relu)
                        rbv = rb[:].rearrange("p (t a) -> p t a", a=64)
                        pos = relup.tile([128, 8], fp32, tag=f"pos{bt}")
                        nc.vector.tensor_reduce(
                            pos[:, 0:nsb], rbv[:, 0:nsb, 0:npos], axis=AX.X, op=OP.add
                        )
                        neg = relup.tile([128, 8], fp32, tag=f"neg{bt}")
                        nc.vector.tensor_reduce(
                            neg[:, 0:nsb], rbv[:, 0:nsb, npos:64], axis=AX.X, op=OP.add
                        )
                        nc.vector.tensor_tensor(
                            scores[bt][:, base : base + nsb], pos[:, 0:nsb], neg[:, 0:nsb],
                            OP.subtract,
                        )

            # =========== PHASE 3 tail: mask + softmax + att^T to DRAM ===========
            for bt in range(BT):
                iob = ew.tile([128, S], fp32, tag="iob")
                nc.gpsimd.partition_broadcast(iob[:], iota_s[0:1, :])
                negb = ew.tile([128, S], fp32, tag="negb")
                nc.vector.memset(negb[:], -1e9)
                pen = ew.tile([128, S], fp32, tag="pen")
                nc.vector.scalar_tensor_tensor(
                    pen[:], iob[:], lens_s[:, bt : bt + 1], negb[:],
                    op0=OP.is_ge, op1=OP.mult,
                )
                nc.vector.tensor_tensor(scores[bt][:], scores[bt][:], pen[:], OP.add)
                mx = ew.tile([128, 1], fp32, tag="mx")
                nc.vector.tensor_reduce(mx[:], scores[bt][:], axis=AX.X, op=OP.max, negate=True)
                ex = ew.tile([128, S], fp32, tag="ex")
                sm = ew.tile([128, 1], fp32, tag="sm")
                nc.scalar.activation(ex[:], scores[bt][:], AF.Exp, bias=mx[:], accum_out=sm[:])
                rcp = ew.tile([128, 1], fp32, tag="rcp")
                nc.vector.reciprocal(rcp[:], sm[:])
                aw = ew.tile([128, S], f32r, tag="aw")
                nc.vector.tensor_scalar_mul(aw[:], ex[:], rcp[:])
                for c0, cn in ((0, 128), (128, S - 128)):
                    pat = psA.tile([128, BS], fp32, tag="phn0", name="pat")
                    nc.tensor.transpose(pat[0:cn, 0:128].bitcast(f32r),
                                        aw[:, c0 : c0 + cn], idr_s[:])
                    sat = stage.tile([128, 128], fp32, tag="sat")
                    nc.scalar.copy(sat[0:cn, :], pat[0:cn, 0:128])
                    nc.sync.dma_start(
                        att_d[c0 : c0 + cn, bt * 128 : (bt + 1) * 128], sat[0:cn, :]
                    )

            # =========== PHASE 4: attentional GRU scan ===========
            h4 = [cpool.tile([H, BS], bf16, tag=f"h4_{i}", name=f"h4_{i}") for i in range(2)]
            nc.vector.memset(h4[0][:], 0.0)
            nc.vector.memset(h4[1][:], 0.0)
            hi = None
            ar = None
            for s in range(S):
                c = widths[s]
                hp4 = h4[s % 2]
                hn4 = h4[(s + 1) % 2]

                if s % 8 == 0:
                    ns = min(8, S - s)
                    cb = c
                    hi = hip.tile([H, 8 * BS], bf16, tag="hi")
                    nc.sync.dma_start(
                        hi[:, 0 : ns * BS].rearrange("h (s b) -> h s b", b=BS)[:, :, 0:cb],
                        outs_d[s : s + ns, :, 0:cb].rearrange("s h b -> h s b"),
                    )
                    ar = hip.tile([1, 8 * BS], fp32, tag="ar")
                    nc.sync.dma_start(
                        ar[:, 0 : ns * BS].rearrange("o (s b) -> o s b", b=BS)[:, :, 0:cb],
                        att_d[s : s + ns, 0:cb].rearrange("(o s) b -> o s b", o=1),
                    )
                hi_s = hi[:, (s % 8) * BS : (s % 8) * BS + c]
                a_row = ar[:, (s % 8) * BS : (s % 8) * BS + c]

                # attention-weight broadcast via PE (off-path)
                p_ab = psA.tile([128, BS], fp32, tag="phn1", name="pab")
                nc.tensor.matmul(
                    p_ab[0:H, 0:c], ones_s[:], a_row.bitcast(f32r),
                    start=True, stop=True, tile_position=(0, 0),
                )

                # z and r gates in separate 64-part PSUM tiles (aligned rows 0:64)
                p_z = psA.tile([128, BS], fp32, tag="prz", name="pz")
                nc.tensor.matmul(
                    p_z[0:H, 0:c], w4i_s[:, 0:64], hi_s, start=True, stop=False,
                    tile_position=(0, 0),
                )
                nc.tensor.matmul(
                    p_z[0:H, 0:c], w4h_s[:, 0:64], hp4[:, 0:c], start=False, stop=True,
                    tile_position=(0, 0),
                )
                p_r = psA.tile([128, BS], fp32, tag="phn0", name="pr")
                nc.tensor.matmul(
                    p_r[0:H, 0:c], w4i_s[:, 64:128], hi_s, start=True, stop=False,
                    tile_position=(0, 0),
                )
                nc.tensor.matmul(
                    p_r[0:H, 0:c], w4h_s[:, 64:128], hp4[:, 0:c], start=False, stop=True,
                    tile_position=(0, 0),
                )
                r_t = ew.tile([H, BS], fp32, tag="rt4", name="rt4")
                nc.scalar.activation(r_t[:, 0:c], p_r[0:H, 0:c], AF.Sigmoid, bias=bias4_s[:, 1:2])
                z_t = ew.tile([H, BS], fp32, tag="zt4", name="zt4")
                nc.scalar.activation(z_t[:, 0:c], p_z[0:H, 0:c], AF.Sigmoid, bias=bias4_s[:, 0:1])

                # zb = z*a ; off-path q = zb*h, p = h - q
                zb = ew.tile([H, BS], fp32, tag="zb4", name="zb4")
                nc.vector.tensor_tensor(zb[:, 0:c], z_t[:, 0:c], p_ab[0:H, 0:c], OP.mult)
                q4 = ew.tile([H, BS], fp32, tag="q4", name="q4")
                nc.vector.tensor_tensor(q4[:, 0:c], zb[:, 0:c], hp4[:, 0:c], OP.mult)
                pp4 = ew.tile([H, BS], fp32, tag="pp4", name="pp4")
                nc.vector.tensor_tensor(pp4[:, 0:c], hp4[:, 0:c], q4[:, 0:c], OP.subtract)

                # on-path: rh = r*h -> n matmul accum -> tanh
                rh = ew.tile([H, BS], bf16, tag="rh4", name="rh4")
                nc.vector.tensor_tensor(rh[:, 0:c], r_t[:, 0:c], hp4[:, 0:c], OP.mult)
                p_n4 = psA.tile([H, BS], fp32, tag="pn", name="pn4")
                nc.tensor.matmul(
                    p_n4[:, 0:c], w4i_s[:, 128:192], hi_s, start=True, stop=False,
                    tile_position=(0, 0),
                )
                nc.tensor.matmul(
                    p_n4[:, 0:c], w4h_s[:, 128:192], rh[:, 0:c], start=False, stop=True,
                    tile_position=(0, 0),
                )
                n4 = ew.tile([H, BS], fp32, tag="n4", name="n4")
                nc.scalar.activation(n4[:, 0:c], p_n4[:, 0:c], AF.Tanh, bias=bias4_s[:, 2:3])

                # on-path combine: w = zb*n ; h' = p + w
                w4 = ew.tile([H, BS], fp32, tag="w4", name="w4")
                nc.vector.tensor_tensor(w4[:, 0:c], zb[:, 0:c], n4[:, 0:c], OP.mult)
                nc.vector.tensor_tensor(hn4[:, 0:c], pp4[:, 0:c], w4[:, 0:c], OP.add)

                # keep freezing columns coherent across the ping-pong pair
                cnext = widths[s + 1] if s + 1 < S else 0
                if cnext < c:
                    nc.gpsimd.tensor_copy(out=hp4[:, cnext:c], in_=hn4[:, cnext:c])

            # =========== epilogue: final h4 -> DRAM (feature-major) ===========
            hfin = h4[S % 2]
            nc.sync.dma_start(hout[:], hfin[:])

    nc.finalize()
    return nc


def _prep_host_inputs(inputs):
    behavior = np.ascontiguousarray(np.asarray(inputs["behavior"], dtype=np.float32))
    target = np.ascontiguousarray(np.asarray(inputs["target"], dtype=np.float32))
    lengths_i = np.asarray(inputs["lengths"]).astype(np.int64).reshape(B)
    Wih = np.asarray(inputs["Wih"], dtype=np.float32)
    Whh = np.asarray(inputs["Whh"], dtype=np.float32)
    bih = np.asarray(inputs["bih"], dtype=np.float32)
    bhh = np.asarray(inputs["bhh"], dtype=np.float32)
    A1 = np.asarray(inputs["A1"], dtype=np.float32)
    b1 = np.asarray(inputs["b1"], dtype=np.float32)
    A2 = np.asarray(inputs["A2"], dtype=np.float32).reshape(-1)
    Wr = np.asarray(inputs["Wr"], dtype=np.float32)
    Wz = np.asarray(inputs["Wz"], dtype=np.float32)
    Wn = np.asarray(inputs["Wn"], dtype=np.float32)
    br = np.asarray(inputs["br"], dtype=np.float32)
    bz = np.asarray(inputs["bz"], dtype=np.float32)
    bn = np.asarray(inputs["bn"], dtype=np.float32)

    assert not np.any(b1), "nonzero b1 not supported by this kernel build"

    # Global sort by length (descending), round-robin shard across cores so
    # every core sees ~the same per-step active-width profile.
    order = np.argsort(-lengths_i, kind="stable")
    core_idx = [order[k::NCORES] for k in range(NCORES)]

    # per-step active width (max over cores), rounded up to 32
    c_per_core = np.stack([
        (np.asarray(lengths_i[idx])[None, :] > np.arange(S)[:, None]).sum(axis=1)
        for idx in core_idx
    ])  # [NCORES, S]
    cmax = c_per_core.max(axis=0)
    widths = np.clip(((cmax + 31) // 32) * 32, 32, BS).astype(int)
    widths = np.maximum.accumulate(widths[::-1])[::-1]  # enforce non-increasing
    widths = tuple(int(w) for w in widths)

    # ph2 gate column order [z | r | n]: z's sigma output lands on
    # partitions 0:64 (used by the h-update), r on 64:128 (n-gate path).
    perm = np.concatenate([np.arange(64, 128), np.arange(0, 64), np.arange(128, 192)])
    wihT = np.concatenate([Wih.T[:, perm], Wih.T[:, perm]], axis=0).astype(bfloat16)
    whhT = np.concatenate([Whh.T[:, perm], Whh.T[:, perm]], axis=0).astype(bfloat16)

    a2order = np.argsort(~(A2 > 0), kind="stable")
    npos = int((A2 > 0).sum())
    A1s = (np.abs(A2)[:, None] * A1)[a2order]
    a1sT = np.ascontiguousarray(A1s.T).astype(bfloat16)

    # AGRU gate order [z | r | n]
    w4iT = np.concatenate([Wz[:, 0:H].T, Wr[:, 0:H].T, Wn[:, 0:H].T], axis=1).astype(bfloat16)
    w4hT = np.concatenate([Wz[:, H:].T, Wr[:, H:].T, Wn[:, H:].T], axis=1).astype(bfloat16)

    identr = np.eye(128, dtype=np.float32)
    identb = np.eye(128, dtype=np.float32).astype(bfloat16)
    iota_r = np.arange(S, dtype=np.float32).reshape(1, S)
    ones64 = np.ones((1, H), np.float32)
    # sigma arg = svec*u + bias2 ; rows 0:64 are z (negated -> 1-z), rows 64:128 are r
    svec = np.concatenate([-np.ones(64, np.float32), np.ones(64, np.float32)]).reshape(128, 1)
    g2 = bih[0:128] + bhh[0:128]
    bias2 = np.concatenate([-g2[64:128], g2[0:64]]).reshape(128, 1).astype(np.float32)
    biasn = np.zeros((128, 2), np.float32)
    biasn[0:64, 0] = bih[128:192]
    biasn[64:128, 1] = bhh[128:192]
    bias4 = np.zeros((H, 3), np.float32)
    bias4[:, 0] = bz
    bias4[:, 1] = br
    bias4[:, 2] = bn

    shared = dict(
        wihT=np.ascontiguousarray(wihT), whhT=np.ascontiguousarray(whhT),
        a1sT=a1sT,
        w4iT=np.ascontiguousarray(w4iT), w4hT=np.ascontiguousarray(w4hT),
        identr=identr, identb=identb, iota_r=iota_r, ones64=ones64, svec=svec,
        bias2=bias2, biasn=biasn, bias4=bias4,
    )
    in_maps = []
    for k in range(NCORES):
        idx = core_idx[k]
        m = dict(shared)
        m["behavior"] = np.ascontiguousarray(behavior[idx])
        m["target"] = np.ascontiguousarray(target[idx])
        m["lengths_f"] = np.ascontiguousarray(
            lengths_i[idx].astype(np.float32).reshape(BS, 1))
        in_maps.append(m)
    return in_maps, npos, widths, core_idx


def kernel(**inputs) -> np.ndarray:
    from concourse.bass_utils import run_bass_kernel_spmd

    in_maps, npos, widths, core_idx = _prep_host_inputs(inputs)
    key = (npos, widths)
    if key not in _CACHE:
        _CACHE[key] = _build_program(npos, widths)
    nc = _CACHE[key]

    trace = os.environ.get("DIEN_TRACE", "0") == "1"
    res = run_bass_kernel_spmd(nc, in_maps, core_ids=list(range(NCORES)), trace=trace)
    out = np.zeros((B, H), np.float32)
    for k in range(NCORES):
        out[core_idx[k]] = res.results[k]["h_out"].astype(np.float32).T
    kernel._last_exec_time_ns = res.exec_time_ns
    return out
